# revision 19
# baseline (speedup 1.0000x reference)
"""Bass/Trainium2 kernel for a 2-layer bidirectional LSTM + linear head.

Problem: x (S=2048, B=64, I=64) -> bilstm(2 layers, H=128, bidir) -> linear(256->1)
Sharding: data-parallel over batch (8 cores x 8 batch each). Weights replicated.

Per-core design (latency-oriented; the wall clock is 4096 serial recurrence
steps, so the kernel minimizes the per-step dependency chain):
  - Hidden dim H=128 on SBUF partitions; batch (8) on the free axis.
  - Forward and backward directions run as two INDEPENDENT instruction
    chains (separate PSUM tiles / activations), interleaved so each chain's
    latency hides under the other's engine work.
  - All-sigmoid gates: the g-gate rows of every weight/bias are pre-scaled
    by 2, so one Sigmoid activation over [128, 32] (i|f|o|g x 8 batch)
    yields sigma(i), sigma(f), sigma(o), and sigma(2g) = (tanh(g)+1)/2.
  - Cell state kept as d = 2c:  d_new = sig(f) * d_old + 4*(sig(2g)-0.5)*sig(i)
    (two scalar_tensor_tensor + one tensor_mul on the Vector engine).
  - tanh(c) = 2*sigma(d)-1 via a second small Sigmoid; h is stored as
    h/2 = (sigma(d)-0.5)*sigma(o) in ONE fused scalar_tensor_tensor.
    The h/2 scaling is compensated by doubling W_hh, layer-1 W_ih and w_out.
  - Input contributions gx are written blockwise by big matmuls DIRECTLY
    INTO the per-block PSUM tiles (gate-major layout [128, 4, BLK, 8]); the
    per-step W_hh matmuls then accumulate onto them with start=False. No
    identity-inject matmul, no PSUM->SBUF gx copies, no gx SBUF storage.
    Layer-0 bias rides a ones-row appended to xT (65 contract rows);
    layer-1 bias is a rank-1 (contract=1) outer-product matmul.
  - Layer-1 backward gx blocks are produced in reverse time order so the
    matmul inputs stay contiguous; consumption indexes them backwards.
  - All matmul operands bf16 (fp32 PSUM accumulation); cell state fp32.
"""

import numpy as np
import ml_dtypes

S, B, I, H = 2048, 64, 64, 128
NCORES = 8
BC = B // NCORES            # batch per core = 8
BLK = 16                    # gx block size (steps); [128,4,BLK,8] f32 = 1 bank
BF16 = ml_dtypes.bfloat16

# slot order in gate tiles: [i, f, o, g]; pytorch row order is [i, f, g, o]
_GATE_ROWS = [(0, 128), (128, 256), (384, 512), (256, 384)]  # i, f, o, g


def _build_program(s_len):
    """Build the Bass program (same for every core). Returns nc."""
    import concourse.bass as bass
    import concourse.tile as tile
    from concourse import bacc, mybir
    from contextlib import ExitStack
    from collections import deque

    bf = mybir.dt.bfloat16
    f32 = mybir.dt.float32
    Act = mybir.ActivationFunctionType
    Alu = mybir.AluOpType

    NB = s_len * BC            # h columns (time-major, batch minor)
    NBLK = s_len // BLK
    assert s_len % BLK == 0

    nc = bacc.Bacc("TRN2", debug=False, enable_asserts=False)

    # ---- DRAM parameters ----
    xT_d = nc.dram_tensor("xT", [65, NB], bf, kind="ExternalInput")
    wih0_d = nc.dram_tensor("wih0", [65, 1024], bf, kind="ExternalInput")
    whh_d = nc.dram_tensor("whh", [128, 2048], bf, kind="ExternalInput")
    wih1_d = nc.dram_tensor("wih1", [128, 2048], bf, kind="ExternalInput")
    bias1_d = nc.dram_tensor("bias1", [1, 1024], bf, kind="ExternalInput")
    ones_d = nc.dram_tensor("ones", [1, BLK * BC], bf, kind="ExternalInput")
    wout_d = nc.dram_tensor("wout", [128, 2], bf, kind="ExternalInput")
    bout_d = nc.dram_tensor("bout", [1, 1], f32, kind="ExternalInput")
    y_d = nc.dram_tensor("y", [1, NB], f32, kind="ExternalOutput")

    with tile.TileContext(nc) as tc, ExitStack() as ctx:
        const = ctx.enter_context(tc.tile_pool(name="const", bufs=1))

        wih0_sb = const.tile([65, 1024], bf)
        nc.sync.dma_start(wih0_sb[:], wih0_d[:])
        whh_sb = const.tile([128, 2048], bf)
        nc.sync.dma_start(whh_sb[:], whh_d[:])
        wih1_sb = const.tile([128, 2048], bf)
        nc.sync.dma_start(wih1_sb[:], wih1_d[:])
        bias1_sb = const.tile([1, 1024], bf)
        nc.sync.dma_start(bias1_sb[:], bias1_d[:])
        ones_sb = const.tile([1, BLK * BC], bf)
        nc.sync.dma_start(ones_sb[:], ones_d[:])
        wout_sb = const.tile([128, 2], bf)
        nc.sync.dma_start(wout_sb[:], wout_d[:])
        bout_sb = const.tile([1, 1], f32)
        nc.sync.dma_start(bout_sb[:], bout_d[:])

        hpool0 = ctx.enter_context(tc.tile_pool(name="h0", bufs=1))
        hf0 = hpool0.tile([128, NB], bf)
        hb0 = hpool0.tile([128, NB], bf)

        dpool = ctx.enter_context(tc.tile_pool(name="dp", bufs=1))
        spool = ctx.enter_context(tc.tile_pool(name="sp", bufs=4))
        gxpool = [ctx.enter_context(
            tc.tile_pool(name=f"gx{c}", bufs=2, space="PSUM"))
            for c in range(2)]

        def make_gx_granules(w, c, j, gxtiles):
            """Emit-granules producing gx block j for chain c of wave w into
            a PSUM tile [128, 4(gate), BLK, 8], forward time order. Chain F:
            block j = steps [j*BLK, (j+1)*BLK); chain B: times
            [s_len-(j+1)*BLK, s_len-j*BLK) (consumed in reverse)."""
            if c == 0:
                c0 = j * BLK * BC
            else:
                c0 = (s_len - (j + 1) * BLK) * BC
            ncols = BLK * BC

            def alloc(_c=c, _j=j):
                gxtiles[_c][_j] = gxpool[_c].tile(
                    [128, 4, BLK, BC], f32, name=f"gxb{_c}")

            grans = [alloc]
            for gate in range(4):
                def mm(gate=gate):
                    # start=True only on the block's first matmul: it arms
                    # lazy-zero for the whole 2KB PSUM bank; every later
                    # write (production or per-step W_hh) uses start=False.
                    # The group is stopped by the last step's W_hh matmul.
                    gxb = gxtiles[c][j]
                    out = gxb[:, gate]
                    if w == 0:
                        nc.tensor.matmul(
                            out,
                            wih0_sb[:, (c * 4 + gate) * 128:
                                    (c * 4 + gate + 1) * 128],
                            xT_sb[:, c0:c0 + ncols],
                            start=(gate == 0), stop=False,
                            skip_group_check=True)
                    else:
                        base = ((c * 4 + gate) * 2) * 128
                        nc.tensor.matmul(
                            out, wih1_sb[:, base:base + 128],
                            hf0[:, c0:c0 + ncols],
                            start=(gate == 0), stop=False,
                            skip_group_check=True)
                        nc.tensor.matmul(
                            out, wih1_sb[:, base + 128:base + 256],
                            hb0[:, c0:c0 + ncols], start=False, stop=False,
                            skip_group_check=True)
                        nc.tensor.matmul(
                            out,
                            bias1_sb[:, (c * 4 + gate) * 128:
                                     (c * 4 + gate + 1) * 128],
                            ones_sb[:], start=False, stop=False,
                            skip_group_check=True)
                grans.append(mm)
            return grans

        def run_wave(w, hf_w, hb_w):
            # Two independent chains (F, B), batch-8 each. Filler
            # instructions (reading ones_sb, writing scratch) are placed in
            # front of cross-engine-dependent ops so they dispatch after
            # their semaphore already landed (blocked dispatch costs ~90-130
            # ns extra on both Scalar and Vector engines).
            d = [dpool.tile([128, 8], f32, name=f"d{c}") for c in range(2)]
            for c in range(2):
                nc.vector.memset(d[c][:], 0.0)

            gxtiles = [{}, {}]
            queues = [deque(), deque()]
            # block 0 produced upfront
            for c in range(2):
                for g in make_gx_granules(w, c, 0, gxtiles):
                    g()
            hists = [hf_w, hb_w]
            A = [None, None]
            T = [None, None]
            for k in range(s_len):
                j, kb = divmod(k, BLK)
                if kb == 0 and j + 1 < NBLK:
                    for c in range(2):
                        queues[c].extend(
                            make_gx_granules(w, c, j + 1, gxtiles))
                if kb == 0 and j >= 2:
                    for c in range(2):
                        gxtiles[c].pop(j - 2, None)

                for c in range(2):
                    pos = kb if c == 0 else BLK - 1 - kb
                    gxb = gxtiles[c][j]
                    if k > 0:
                        prev = (k - 1) if c == 0 else (s_len - k)
                        rhs = hists[c][:, prev * BC:(prev + 1) * BC]
                        for gi in range(4):
                            nc.tensor.matmul(
                                gxb[:, gi, pos, :],
                                whh_sb[:, ((w * 2 + c) * 4 + gi) * 128:
                                       ((w * 2 + c) * 4 + gi + 1) * 128],
                                rhs, start=False, stop=False,
                                skip_group_check=True)
                    if queues[c]:
                        queues[c].popleft()()

                for c in range(2):
                    pos = kb if c == 0 else BLK - 1 - kb
                    gxb = gxtiles[c][j]
                    A[c] = spool.tile([128, 32], f32, name=f"A{c}")
                    nc.scalar.activation(A[c][:], gxb[:, :, pos, :],
                                         Act.Sigmoid)

                for c in range(2):
                    vt = spool.tile([128, 8], f32, name=f"v{c}")
                    nc.vector.tensor_mul(vt[:], A[c][:, 8:16], d[c][:])
                    ut = spool.tile([128, 8], f32, name=f"u{c}")
                    nc.vector.scalar_tensor_tensor(
                        ut[:], A[c][:, 24:32], 0.5, A[c][:, 0:8],
                        Alu.subtract, Alu.mult)
                    nc.vector.scalar_tensor_tensor(
                        d[c][:], ut[:], 4.0, vt[:], Alu.mult, Alu.add)

                for c in range(2):
                    T[c] = spool.tile([128, 8], f32, name=f"T{c}")
                    nc.scalar.activation(T[c][:], d[c][:], Act.Sigmoid)

                for c in range(2):
                    t_out = k if c == 0 else s_len - 1 - k
                    nc.vector.scalar_tensor_tensor(
                        hists[c][:, t_out * BC:(t_out + 1) * BC],
                        T[c][:], 0.5, A[c][:, 16:24],
                        Alu.subtract, Alu.mult)

        # ---- wave 0 (layer 0); x staged in a scoped pool so SBUF is reused
        with tc.tile_pool(name="xp", bufs=1) as xpool:
            xT_sb = xpool.tile([65, NB], bf)
            nc.sync.dma_start(xT_sb[:], xT_d[:])
            run_wave(0, hf0, hb0)

        # ---- wave 1 (layer 1) ----
        with tc.tile_pool(name="h1", bufs=1) as hpool1:
            hf1 = hpool1.tile([128, NB], bf)
            hb1 = hpool1.tile([128, NB], bf)
            run_wave(1, hf1, hb1)

            # ---- output projection (w_out pre-doubled for the h/2 store) --
            with tc.tile_pool(name="yp", bufs=3) as ypool, \
                 tc.tile_pool(name="pyp", bufs=2, space="PSUM") as pypool:
                for cc in range(NB // 512):
                    py = pypool.tile([1, 512], f32, name="py")
                    nc.tensor.matmul(py[:], wout_sb[:, 0:1],
                                     hf1[:, cc * 512:(cc + 1) * 512],
                                     start=True, stop=False)
                    nc.tensor.matmul(py[:], wout_sb[:, 1:2],
                                     hb1[:, cc * 512:(cc + 1) * 512],
                                     start=False, stop=True)
                    y_sb = ypool.tile([1, 512], f32, name="y_sb")
                    nc.scalar.activation(y_sb[:], py[:], Act.Identity,
                                         bias=bout_sb[0:1, 0:1])
                    nc.sync.dma_start(y_d[0:1, cc * 512:(cc + 1) * 512], y_sb[:])

    nc.compile()
    return nc


def _prep_shared(inputs, s_len):
    """Host-side packing of (replicated) weight tensors.

    Scale conventions (see module docstring): h is stored on-device as h/2,
    and the g-gate uses sigma(2g). Hence:
      W_hh: x2 (h comp), g-gate block x2 more  -> 2 / 4
      W_ih layer0: x1, g rows x2               -> 1 / 2
      W_ih layer1: x2 (h0 comp), g rows x2     -> 2 / 4
      biases: g rows x2
      w_out: x2 (h1 comp)
    """
    def bfc(a):
        return np.ascontiguousarray(a).astype(BF16)

    wih0 = np.zeros((65, 1024), np.float32)
    whh = np.zeros((128, 2048), np.float32)
    wih1 = np.zeros((128, 2048), np.float32)
    bias1 = np.zeros((1, 1024), np.float32)

    w_ih_l0 = [inputs['w_ih_f0'], inputs['w_ih_r0']]
    w_ih_l1 = [inputs['w_ih_f1'], inputs['w_ih_r1']]
    w_hh_l = [[inputs['w_hh_f0'], inputs['w_hh_r0']],
              [inputs['w_hh_f1'], inputs['w_hh_r1']]]
    b_l = [[inputs['b_f0'], inputs['b_r0']], [inputs['b_f1'], inputs['b_r1']]]

    for d in range(2):
        for gi in range(4):
            r0, r1 = _GATE_ROWS[gi]
            gs = 2.0 if gi == 3 else 1.0
            col = d * 4 + gi
            wih0[0:64, col * 128:(col + 1) * 128] = \
                w_ih_l0[d][r0:r1, :].T * gs
            wih0[64, col * 128:(col + 1) * 128] = b_l[0][d][r0:r1] * gs
            bias1[0, col * 128:(col + 1) * 128] = b_l[1][d][r0:r1] * gs
            for half in range(2):
                base = (col * 2 + half) * 128
                wih1[:, base:base + 128] = \
                    w_ih_l1[d][r0:r1, half * 128:(half + 1) * 128].T * (2 * gs)
            for w in range(2):
                wcol = (w * 2 + d) * 4 + gi
                whh[:, wcol * 128:(wcol + 1) * 128] = \
                    w_hh_l[w][d][r0:r1, :].T * (2 * gs)

    wout = np.zeros((128, 2), np.float32)
    wout[:, 0] = inputs['w_out'][0, 0:128] * 2.0
    wout[:, 1] = inputs['w_out'][0, 128:256] * 2.0
    bout = np.asarray(inputs['b_out'], np.float32).reshape(1, 1)

    return {
        'wih0': bfc(wih0), 'whh': bfc(whh), 'wih1': bfc(wih1),
        'bias1': bfc(bias1), 'ones': bfc(np.ones((1, BLK * BC), np.float32)),
        'wout': bfc(wout), 'bout': bout,
    }


def _prep_core(x, core, s_len):
    cb = core * BC
    xs = np.asarray(x[:s_len, cb:cb + BC, :], np.float32)
    xT = np.empty((65, s_len * BC), np.float32)
    xT[0:64] = xs.transpose(2, 0, 1).reshape(64, s_len * BC)
    xT[64] = 1.0
    return {'xT': xT.astype(BF16)}


_CACHED = {}


def _get_program(s_len):
    if s_len not in _CACHED:
        _CACHED[s_len] = _build_program(s_len)
    return _CACHED[s_len]


def kernel(**inputs):
    from concourse.bass_utils import run_bass_kernel_spmd

    x = np.asarray(inputs['x'], np.float32)
    s_len = x.shape[0]
    nc = _get_program(s_len)
    shared = _prep_shared(inputs, s_len)
    in_maps = [dict(shared, **_prep_core(x, c, s_len)) for c in range(NCORES)]
    res = run_bass_kernel_spmd(nc, in_maps, list(range(NCORES)))
    outs = []
    for c in range(NCORES):
        yc = np.asarray(res.results[c]['y']).reshape(s_len, BC)
        outs.append(yc)
    y = np.concatenate(outs, axis=1)[:, :, None].astype(np.float32)
    return y


# revision 20
# speedup vs baseline: 1.1913x; 1.1913x over previous
"""Bass/Trainium2 kernel for a 2-layer bidirectional LSTM + linear head.

Problem: x (S=2048, B=64, I=64) -> bilstm(2 layers, H=128, bidir) -> linear(256->1)
Sharding: data-parallel over batch (8 cores x 8 batch each). Weights replicated.

Per-core design (latency-oriented; the wall clock is 4096 serial recurrence
steps, so the kernel minimizes the per-step dependency chain):
  - Hidden dim H=128 on SBUF partitions; batch (8) on the free axis.
  - Forward and backward directions run as two INDEPENDENT instruction
    chains (separate PSUM tiles / activations), interleaved so each chain's
    latency hides under the other's engine work.
  - All-sigmoid gates: the g-gate rows of every weight/bias are pre-scaled
    by 2, so one Sigmoid activation over [128, 32] (i|f|o|g x 8 batch)
    yields sigma(i), sigma(f), sigma(o), and sigma(2g) = (tanh(g)+1)/2.
  - Cell state kept as d = 2c:  d_new = sig(f) * d_old + 4*(sig(2g)-0.5)*sig(i)
    (two scalar_tensor_tensor + one tensor_mul on the Vector engine).
  - tanh(c) = 2*sigma(d)-1 via a second small Sigmoid; h is stored as
    h/2 = (sigma(d)-0.5)*sigma(o) in ONE fused scalar_tensor_tensor.
    The h/2 scaling is compensated by doubling W_hh, layer-1 W_ih and w_out.
  - Input contributions gx are written blockwise by big matmuls DIRECTLY
    INTO the per-block PSUM tiles (gate-major layout [128, 4, BLK, 8]); the
    per-step W_hh matmuls then accumulate onto them with start=False. No
    identity-inject matmul, no PSUM->SBUF gx copies, no gx SBUF storage.
    Layer-0 bias rides a ones-row appended to xT (65 contract rows);
    layer-1 bias is a rank-1 (contract=1) outer-product matmul.
  - Layer-1 backward gx blocks are produced in reverse time order so the
    matmul inputs stay contiguous; consumption indexes them backwards.
  - All matmul operands bf16 (fp32 PSUM accumulation); cell state fp32.
"""

import numpy as np
import ml_dtypes

S, B, I, H = 2048, 64, 64, 128
NCORES = 8
BC = B // NCORES            # batch per core = 8
BLK = 16                    # gx block size (steps); [128,4,BLK,8] f32 = 1 bank
BF16 = ml_dtypes.bfloat16

# slot order in gate tiles: [i, f, o, g]; pytorch row order is [i, f, g, o]
_GATE_ROWS = [(0, 128), (128, 256), (384, 512), (256, 384)]  # i, f, o, g


def _build_program(s_len):
    """Build the Bass program (same for every core). Returns nc."""
    import concourse.bass as bass
    import concourse.tile as tile
    from concourse import bacc, mybir
    from contextlib import ExitStack
    from collections import deque

    bf = mybir.dt.bfloat16
    f32 = mybir.dt.float32
    Act = mybir.ActivationFunctionType
    Alu = mybir.AluOpType

    NB = s_len * BC            # h columns (time-major, batch minor)
    NBLK = s_len // BLK
    assert s_len % BLK == 0

    nc = bacc.Bacc("TRN2", debug=False, enable_asserts=False)

    # ---- DRAM parameters ----
    xT_d = nc.dram_tensor("xT", [65, NB], bf, kind="ExternalInput")
    wih0_d = nc.dram_tensor("wih0", [65, 1024], bf, kind="ExternalInput")
    whh_d = nc.dram_tensor("whh", [128, 2048], bf, kind="ExternalInput")
    wih1_d = nc.dram_tensor("wih1", [128, 2048], bf, kind="ExternalInput")
    bias1_d = nc.dram_tensor("bias1", [1, 1024], bf, kind="ExternalInput")
    ones_d = nc.dram_tensor("ones", [1, BLK * BC], bf, kind="ExternalInput")
    wout_d = nc.dram_tensor("wout", [128, 2], bf, kind="ExternalInput")
    bout_d = nc.dram_tensor("bout", [1, 1], f32, kind="ExternalInput")
    y_d = nc.dram_tensor("y", [1, NB], f32, kind="ExternalOutput")

    with tile.TileContext(nc) as tc, ExitStack() as ctx:
        const = ctx.enter_context(tc.tile_pool(name="const", bufs=1))

        wih0_sb = const.tile([65, 1024], bf)
        nc.sync.dma_start(wih0_sb[:], wih0_d[:])
        whh_sb = const.tile([128, 2048], bf)
        nc.sync.dma_start(whh_sb[:], whh_d[:])
        wih1_sb = const.tile([128, 2048], bf)
        nc.sync.dma_start(wih1_sb[:], wih1_d[:])
        bias1_sb = const.tile([1, 1024], bf)
        nc.sync.dma_start(bias1_sb[:], bias1_d[:])
        ones_sb = const.tile([1, BLK * BC], bf)
        nc.sync.dma_start(ones_sb[:], ones_d[:])
        wout_sb = const.tile([128, 2], bf)
        nc.sync.dma_start(wout_sb[:], wout_d[:])
        bout_sb = const.tile([1, 1], f32)
        nc.sync.dma_start(bout_sb[:], bout_d[:])

        hpool0 = ctx.enter_context(tc.tile_pool(name="h0", bufs=1))
        hf0 = hpool0.tile([128, NB], bf)
        hb0 = hpool0.tile([128, NB], bf)

        dpool = ctx.enter_context(tc.tile_pool(name="dp", bufs=1))
        spool = ctx.enter_context(tc.tile_pool(name="sp", bufs=4))
        gxpool = [ctx.enter_context(
            tc.tile_pool(name=f"gx{c}", bufs=2, space="PSUM"))
            for c in range(2)]
        pfpool = ctx.enter_context(
            tc.tile_pool(name="pf", bufs=2, space="PSUM"))

        def make_gx_granules(w, c, j, gxtiles):
            """Emit-granules producing gx block j for chain c of wave w into
            a PSUM tile [128, 4(gate), BLK, 8], forward time order. Chain F:
            block j = steps [j*BLK, (j+1)*BLK); chain B: times
            [s_len-(j+1)*BLK, s_len-j*BLK) (consumed in reverse)."""
            if c == 0:
                c0 = j * BLK * BC
            else:
                c0 = (s_len - (j + 1) * BLK) * BC
            ncols = BLK * BC

            def alloc(_c=c, _j=j):
                gxtiles[_c][_j] = gxpool[_c].tile(
                    [128, 4, BLK, BC], f32, name=f"gxb{_c}")

            grans = [alloc]
            for gate in range(4):
                def mm(gate=gate):
                    # start=True only on the block's first matmul: it arms
                    # lazy-zero for the whole 2KB PSUM bank; every later
                    # write (production or per-step W_hh) uses start=False.
                    # The group is stopped by the last step's W_hh matmul.
                    gxb = gxtiles[c][j]
                    out = gxb[:, gate]
                    if w == 0:
                        nc.tensor.matmul(
                            out,
                            wih0_sb[:, (c * 4 + gate) * 128:
                                    (c * 4 + gate + 1) * 128],
                            xT_sb[:, c0:c0 + ncols],
                            start=(gate == 0), stop=False,
                            skip_group_check=True)
                    else:
                        base = ((c * 4 + gate) * 2) * 128
                        nc.tensor.matmul(
                            out, wih1_sb[:, base:base + 128],
                            hf0[:, c0:c0 + ncols],
                            start=(gate == 0), stop=False,
                            skip_group_check=True)
                        nc.tensor.matmul(
                            out, wih1_sb[:, base + 128:base + 256],
                            hb0[:, c0:c0 + ncols], start=False, stop=False,
                            skip_group_check=True)
                        nc.tensor.matmul(
                            out,
                            bias1_sb[:, (c * 4 + gate) * 128:
                                     (c * 4 + gate + 1) * 128],
                            ones_sb[:], start=False, stop=False,
                            skip_group_check=True)
                grans.append(mm)
            return grans

        def run_wave(w, hf_w, hb_w):
            # Two independent chains (F, B), batch-8 each. A fill matmul
            # absorbs each gate group's semaphore wait + SBUF access latency;
            # one anchored scalar fill keeps the Scalar queue busy so
            # sigma(d) of chain F dispatches after its semaphore landed.
            d = [dpool.tile([128, 8], f32, name=f"d{c}") for c in range(2)]
            for c in range(2):
                nc.vector.memset(d[c][:], 0.0)

            def sfill(tag, n, anchor):
                for i in range(n):
                    t = spool.tile([1, 8], f32, name=f"sf{tag}_{i}")
                    nc.scalar.activation(t[:], anchor, Act.Sigmoid)

            gxtiles = [{}, {}]
            queues = [deque(), deque()]
            # block 0 produced upfront
            for c in range(2):
                for g in make_gx_granules(w, c, 0, gxtiles):
                    g()
            hists = [hf_w, hb_w]
            A = [None, None]
            T = [None, None]
            for k in range(s_len):
                j, kb = divmod(k, BLK)
                if kb == 0 and j + 1 < NBLK:
                    for c in range(2):
                        queues[c].extend(
                            make_gx_granules(w, c, j + 1, gxtiles))
                if kb == 0 and j >= 2:
                    for c in range(2):
                        gxtiles[c].pop(j - 2, None)

                for c in range(2):
                    pos = kb if c == 0 else BLK - 1 - kb
                    gxb = gxtiles[c][j]
                    if k > 0:
                        prev = (k - 1) if c == 0 else (s_len - k)
                        rhs = hists[c][:, prev * BC:(prev + 1) * BC]
                        pf = pfpool.tile([128, 1], f32, name="pf")
                        nc.tensor.matmul(
                            pf[:], whh_sb[:, 0:128], rhs[:, 0:1],
                            start=True, stop=True, skip_group_check=True)
                        for gi in range(4):
                            nc.tensor.matmul(
                                gxb[:, gi, pos, :],
                                whh_sb[:, ((w * 2 + c) * 4 + gi) * 128:
                                       ((w * 2 + c) * 4 + gi + 1) * 128],
                                rhs, start=False, stop=False,
                                skip_group_check=True)
                    if queues[c]:
                        queues[c].popleft()()

                for c in range(2):
                    pos = kb if c == 0 else BLK - 1 - kb
                    gxb = gxtiles[c][j]
                    A[c] = spool.tile([128, 32], f32, name=f"A{c}")
                    nc.scalar.activation(A[c][:], gxb[:, :, pos, :],
                                         Act.Sigmoid)

                for c in range(2):
                    vt = spool.tile([128, 8], f32, name=f"v{c}")
                    nc.vector.tensor_mul(vt[:], A[c][:, 8:16], d[c][:])
                    ut = spool.tile([128, 8], f32, name=f"u{c}")
                    nc.vector.scalar_tensor_tensor(
                        ut[:], A[c][:, 24:32], 0.5, A[c][:, 0:8],
                        Alu.subtract, Alu.mult)
                    nc.vector.scalar_tensor_tensor(
                        d[c][:], ut[:], 4.0, vt[:], Alu.mult, Alu.add)

                sfill(0, 1, A[1][0:1, 0:8])
                for c in range(2):
                    T[c] = spool.tile([128, 8], f32, name=f"T{c}")
                    nc.scalar.activation(T[c][:], d[c][:], Act.Sigmoid)

                for c in range(2):
                    t_out = k if c == 0 else s_len - 1 - k
                    nc.vector.scalar_tensor_tensor(
                        hists[c][:, t_out * BC:(t_out + 1) * BC],
                        T[c][:], 0.5, A[c][:, 16:24],
                        Alu.subtract, Alu.mult)

        # ---- wave 0 (layer 0); x staged in a scoped pool so SBUF is reused
        with tc.tile_pool(name="xp", bufs=1) as xpool:
            xT_sb = xpool.tile([65, NB], bf)
            nc.sync.dma_start(xT_sb[:], xT_d[:])
            run_wave(0, hf0, hb0)

        # ---- wave 1 (layer 1) ----
        with tc.tile_pool(name="h1", bufs=1) as hpool1:
            hf1 = hpool1.tile([128, NB], bf)
            hb1 = hpool1.tile([128, NB], bf)
            run_wave(1, hf1, hb1)

            # ---- output projection (w_out pre-doubled for the h/2 store) --
            with tc.tile_pool(name="yp", bufs=3) as ypool, \
                 tc.tile_pool(name="pyp", bufs=2, space="PSUM") as pypool:
                for cc in range(NB // 512):
                    py = pypool.tile([1, 512], f32, name="py")
                    nc.tensor.matmul(py[:], wout_sb[:, 0:1],
                                     hf1[:, cc * 512:(cc + 1) * 512],
                                     start=True, stop=False)
                    nc.tensor.matmul(py[:], wout_sb[:, 1:2],
                                     hb1[:, cc * 512:(cc + 1) * 512],
                                     start=False, stop=True)
                    y_sb = ypool.tile([1, 512], f32, name="y_sb")
                    nc.scalar.activation(y_sb[:], py[:], Act.Identity,
                                         bias=bout_sb[0:1, 0:1])
                    nc.sync.dma_start(y_d[0:1, cc * 512:(cc + 1) * 512], y_sb[:])

    nc.compile()
    return nc


def _prep_shared(inputs, s_len):
    """Host-side packing of (replicated) weight tensors.

    Scale conventions (see module docstring): h is stored on-device as h/2,
    and the g-gate uses sigma(2g). Hence:
      W_hh: x2 (h comp), g-gate block x2 more  -> 2 / 4
      W_ih layer0: x1, g rows x2               -> 1 / 2
      W_ih layer1: x2 (h0 comp), g rows x2     -> 2 / 4
      biases: g rows x2
      w_out: x2 (h1 comp)
    """
    def bfc(a):
        return np.ascontiguousarray(a).astype(BF16)

    wih0 = np.zeros((65, 1024), np.float32)
    whh = np.zeros((128, 2048), np.float32)
    wih1 = np.zeros((128, 2048), np.float32)
    bias1 = np.zeros((1, 1024), np.float32)

    w_ih_l0 = [inputs['w_ih_f0'], inputs['w_ih_r0']]
    w_ih_l1 = [inputs['w_ih_f1'], inputs['w_ih_r1']]
    w_hh_l = [[inputs['w_hh_f0'], inputs['w_hh_r0']],
              [inputs['w_hh_f1'], inputs['w_hh_r1']]]
    b_l = [[inputs['b_f0'], inputs['b_r0']], [inputs['b_f1'], inputs['b_r1']]]

    for d in range(2):
        for gi in range(4):
            r0, r1 = _GATE_ROWS[gi]
            gs = 2.0 if gi == 3 else 1.0
            col = d * 4 + gi
            wih0[0:64, col * 128:(col + 1) * 128] = \
                w_ih_l0[d][r0:r1, :].T * gs
            wih0[64, col * 128:(col + 1) * 128] = b_l[0][d][r0:r1] * gs
            bias1[0, col * 128:(col + 1) * 128] = b_l[1][d][r0:r1] * gs
            for half in range(2):
                base = (col * 2 + half) * 128
                wih1[:, base:base + 128] = \
                    w_ih_l1[d][r0:r1, half * 128:(half + 1) * 128].T * (2 * gs)
            for w in range(2):
                wcol = (w * 2 + d) * 4 + gi
                whh[:, wcol * 128:(wcol + 1) * 128] = \
                    w_hh_l[w][d][r0:r1, :].T * (2 * gs)

    wout = np.zeros((128, 2), np.float32)
    wout[:, 0] = inputs['w_out'][0, 0:128] * 2.0
    wout[:, 1] = inputs['w_out'][0, 128:256] * 2.0
    bout = np.asarray(inputs['b_out'], np.float32).reshape(1, 1)

    return {
        'wih0': bfc(wih0), 'whh': bfc(whh), 'wih1': bfc(wih1),
        'bias1': bfc(bias1), 'ones': bfc(np.ones((1, BLK * BC), np.float32)),
        'wout': bfc(wout), 'bout': bout,
    }


def _prep_core(x, core, s_len):
    cb = core * BC
    xs = np.asarray(x[:s_len, cb:cb + BC, :], np.float32)
    xT = np.empty((65, s_len * BC), np.float32)
    xT[0:64] = xs.transpose(2, 0, 1).reshape(64, s_len * BC)
    xT[64] = 1.0
    return {'xT': xT.astype(BF16)}


_CACHED = {}


def _get_program(s_len):
    if s_len not in _CACHED:
        _CACHED[s_len] = _build_program(s_len)
    return _CACHED[s_len]


def kernel(**inputs):
    from concourse.bass_utils import run_bass_kernel_spmd

    x = np.asarray(inputs['x'], np.float32)
    s_len = x.shape[0]
    nc = _get_program(s_len)
    shared = _prep_shared(inputs, s_len)
    in_maps = [dict(shared, **_prep_core(x, c, s_len)) for c in range(NCORES)]
    res = run_bass_kernel_spmd(nc, in_maps, list(range(NCORES)))
    outs = []
    for c in range(NCORES):
        yc = np.asarray(res.results[c]['y']).reshape(s_len, BC)
        outs.append(yc)
    y = np.concatenate(outs, axis=1)[:, :, None].astype(np.float32)
    return y
